# revision 54
# baseline (speedup 1.0000x reference)
"""Multi-head self-attention (B=4, T=2048, C=768, H=12) on 8 trn2 NeuronCores. v19.

Sharding: core c -> batch b=c//2, head-group g=c%2 (6 heads each).
Host sums the 2 partials per batch and adds the bias.

Key mechanisms (evolved v9 -> v19):
  - skew-2, 2-step grouped attention pipeline: [s_{j+2}, s_{j+3}] emitted
    before [c_j, c_{j+1}]; the in-order PE queue never head-of-line blocks
    on exp latency, and 64<->128 tile-config switches (unhidden LDWEIGHTS)
    are halved. The last step preloads the NEXT block's first score pair so
    the ACT/DVE exp streams never drain at block boundaries.
  - one shared PSUM pool (3 x [128,1024] f32 = 6 banks) serves scores,
    projections, v chunks, transposes and outproj; cps gets the other 2.
  - mixed-precision ctx: key-chunk pairs outside S_BF16 run exp->fp8e4
    (ACT exp or DVE Schraudolph-to-int8 bits) and a DoubleRow fp8 matmul
    (double contraction per pass); S_BF16 pairs keep bf16 exp + classic
    matmuls to hold max-rel error ~1.5e-2 < 2e-2. va for fp8 pairs is
    pair-interleaved [128,2,80] (DoubleRow needs 16B-aligned plane stride).
  - exp engine split ACT/DVE by fractional accumulator (~0.34 DVE).
  - x DMAs ride the ACT hardware DMA queue, weights the SP queue (parallel
    streams cut the serial-DMA lead-in); out stores alternate queues.
  - normalize: cu release split DVE/ACT frees the cps bank in ~600ns;
    recip + gpsimd partition-broadcast + muls run off the critical path;
    the last block broadcasts via a K=1 bf16 PE matmul instead (PE is idle
    in the tail) and outproj groups trail their normalize by >=1 block.
"""
import sys
import os

sys.path.insert(0, "/opt/trn_rl_repo")

import numpy as np

P = 128
T = 2048
C = 768
HD = 384          # per-core head columns (6 heads x 64)
D = 64
NT = T // P       # 16 key chunks of 128
KC = C // P       # 6 contraction chunks for C
MC = HD // P      # 3 head pairs
QB = 512          # query block
NQ = T // QB      # 4 query blocks

EXP_A = 128 * 0.125 * float(np.log2(np.e))   # 23.083120654223414
EXP_B = 128 * 127 - 5.5                      # 16250.5 (Schraudolph bias, bf16)
EXP8_A = 8 * 0.125 * float(np.log2(np.e))    # 1.4427 (Schraudolph, fp8e4m3)
EXP8_B = 8 * 7 - 0.46                        # 55.54

# key-chunk PAIRS (of 8) that keep bf16 exp + classic ctx matmuls; the rest
# run exp->fp8e4 and a DoubleRow ctx matmul (2x PE throughput). Limiting the
# fp8 fraction keeps the max-rel error ~1.4e-2 < 2e-2.
S_BF16 = frozenset({1, 4, 6})

# DVE share of exp chunks per phase (fractional accumulator)
F_DVE_LEAD = 0.34    # block (0,0): DVE busy with lead-in casts/copies
F_DVE_MAIN = 0.34    # blocks m=0 u>=1 .. m=1
F_DVE_OUT = 0.25     # m=2 phase: DVE also does outproj copies + norm muls

_cache = {}


def _build(repeat=1):
    import concourse.bacc as bacc
    import concourse.mybir as mybir
    import concourse.tile as tile
    from concourse.masks import make_identity
    from contextlib import ExitStack

    F32 = mybir.dt.float32
    BF16 = mybir.dt.bfloat16
    I16 = mybir.dt.int16
    F8 = mybir.dt.float8e4
    I8 = mybir.dt.int8
    AF = mybir.ActivationFunctionType
    ALU = mybir.AluOpType
    DROW = mybir.MatmulPerfMode.DoubleRow

    nc = bacc.Bacc("TRN2", target_bir_lowering=False, debug=False)
    x = nc.dram_tensor("x", [T, C], F32, kind="ExternalInput").ap()
    wq = nc.dram_tensor("wq", [C, HD], F32, kind="ExternalInput").ap()
    wk = nc.dram_tensor("wk", [C, HD], F32, kind="ExternalInput").ap()
    wv = nc.dram_tensor("wv", [C, HD], F32, kind="ExternalInput").ap()
    wo = nc.dram_tensor("wo", [HD, C], F32, kind="ExternalInput").ap()
    out = nc.dram_tensor("out", [T, C], F32, kind="ExternalOutput").ap()

    def emit(pfx, tc, pools):
        ident_bf, big, wrp, vap, work, outp, norm = pools

        xt = [big.tile([P, T], BF16, name=f"{pfx}xt{kc}", tag="big2048") for kc in range(KC)]
        qT = [big.tile([P, T], BF16, name=f"{pfx}qT{m}", tag="big2048") for m in range(MC)]
        kT = [big.tile([P, T], BF16, name=f"{pfx}kT{m}", tag="big2048") for m in range(MC)]
        ctxT = [big.tile([P, T], BF16, name=f"{pfx}ctxT{m}", tag="big2048") for m in range(MC)]
        # v tiles: bf16 pairs keep the classic [P, D+1] per-chunk layout;
        # fp8 pairs hold both chunks of the pair interleaved on the middle
        # dim, padded to 80 cols (DoubleRow needs a 16B-aligned plane stride)
        va = [[[(vap.tile([P, D + 1], BF16, name=f"{pfx}va{m}_{t}_{g}",
                          tag=f"va{m}_{t}_{g}") if (t // 2) in S_BF16 else None)
                for g in range(2)] for t in range(NT)] for m in range(MC)]
        va8 = [[[(None if r in S_BF16 else
                  vap.tile([P, 2, 80], F8, name=f"{pfx}va8{m}_{r}_{g}",
                           tag=f"va8{m}_{r}_{g}"))
                 for g in range(2)] for r in range(NT // 2)] for m in range(MC)]

        w_b = {}
        wo_b = []
        exp_acc = [0.0]
        ones1 = norm.tile([1, D], F32, name=f"{pfx}ones1", tag="ones1")
        nc.gpsimd.memset(ones1[:], 1.0)
        ones_b = norm.tile([1, D], BF16, name=f"{pfx}onesb", tag="onesb")
        nc.vector.tensor_copy(ones_b[:], ones1[:])

        def emit_weight_loads(wstage, names):
            for nm, src in names:
                if nm == "o":
                    for m in range(MC):
                        st = wstage.tile([P, C], F32, name=f"{pfx}wst_o{m}", tag="wsto")
                        nc.sync.dma_start(st[:], wo[P * m:P * (m + 1), :])
                        t_b = wrp.tile([P, C], BF16, name=f"{pfx}wo_{m}", tag=f"wo_{m}")
                        nc.gpsimd.tensor_copy(t_b[:], st[:])
                        wo_b.append(t_b)
                    continue
                for kc in range(KC):
                    st = wstage.tile([P, HD], F32, name=f"{pfx}wst_{nm}{kc}", tag="wst")
                    nc.sync.dma_start(st[:], src[P * kc:P * (kc + 1), :])
                    t_b = wrp.tile([P, HD], BF16, name=f"{pfx}w_{nm}{kc}", tag=f"w_{nm}{kc}")
                    # weight casts must not clog the DVE queue ahead of the
                    # x casts: idle-ACT copies (gpsimd's ~2us DSP casts
                    # stalled the v projections)
                    nc.scalar.copy(t_b[:], st[:])
                    w_b[nm, kc] = t_b

        def emit_xt_group(tq, xrp, xbp, ps):
            xbs = []
            for i in range(4):
                t_i = 4 * tq + i
                xr = xrp.tile([P, C], F32, name=f"{pfx}xr{t_i}", tag="xr")
                # ACT hardware DMA queue: runs parallel to the SP queue that
                # carries the weight loads (serial DMAs cost ~10us lead-in)
                nc.scalar.dma_start(xr[:], x[P * t_i:P * (t_i + 1), :])
                xb = xbp.tile([P, C], BF16, name=f"{pfx}xb{t_i}", tag="xb")
                nc.vector.tensor_copy(xb[:], xr[:])
                xbs.append(xb)
            for kc in range(KC):
                tp = ps.tile([P, 512], BF16, name=f"{pfx}tp_{tq}_{kc}", tag="ps")
                for i in range(4):
                    nc.tensor.transpose(tp[:, P * i:P * (i + 1)],
                                        xbs[i][:, P * kc:P * (kc + 1)], ident_bf[:])
                nc.vector.tensor_copy(xt[kc][:, 512 * tq:512 * (tq + 1)], tp[:])

        def emit_proj_block(nm, m, n, ps):
            dest = qT if nm == "q" else kT
            pst = ps.tile([P, 512], F32, name=f"{pfx}ps_{nm}{m}{n}", tag="ps")
            for kc in range(KC):
                nc.tensor.matmul(
                    pst[:],
                    w_b[nm, kc][:, P * m:P * (m + 1)],
                    xt[kc][:, 512 * n:512 * (n + 1)],
                    start=(kc == 0), stop=(kc == KC - 1),
                )
            if m == 0:
                nc.scalar.copy(dest[m][:, 512 * n:512 * (n + 1)], pst[:])
            else:
                nc.vector.tensor_copy(dest[m][:, 512 * n:512 * (n + 1)], pst[:])

        def emit_v_chunk(t_i, ps):
            pv = ps.tile([P, HD], F32, name=f"{pfx}pv{t_i}", tag="ps")
            for kc in range(KC):
                nc.tensor.matmul(
                    pv[:],
                    xt[kc][:, P * t_i:P * (t_i + 1)],
                    w_b["v", kc][:],
                    start=(kc == 0), stop=(kc == KC - 1),
                )
            r, i = divmod(t_i, 2)
            for m in range(MC):
                for g in range(2):
                    src = pv[:, P * m + D * g:P * m + D * (g + 1)]
                    if r in S_BF16:
                        vt = va[m][t_i][g]
                        nc.vector.tensor_copy(vt[:, 0:D], src)
                        nc.gpsimd.memset(vt[:, D:D + 1], 1.0)
                    else:
                        vt = va8[m][r][g]
                        nc.vector.tensor_copy(vt[:, i, 0:D], src)
                        nc.gpsimd.memset(vt[:, i, D:D + 1], 1.0)

        def scores_mm(m, u, j, ps):
            q0 = QB * u
            sps = ps.tile([P, 2 * QB], F32, name=f"{pfx}sps{m}{u}{j}", tag="ps")
            nc.tensor.matmul(sps[:, 0:QB],
                             kT[m][0:D, P * j:P * (j + 1)],
                             qT[m][0:D, q0:q0 + QB],
                             start=True, stop=True, tile_position=(0, 0))
            nc.tensor.matmul(sps[:, QB:2 * QB],
                             kT[m][D:P, P * j:P * (j + 1)],
                             qT[m][D:P, q0:q0 + QB],
                             start=True, stop=True, tile_position=(64, 0))
            return sps

        def exp_into(sps, dst, f_dve, fp8):
            exp_acc[0] += f_dve
            if exp_acc[0] >= 1.0:
                exp_acc[0] -= 1.0
                if fp8:
                    nc.vector.tensor_scalar(dst.bitcast(I8), sps[:],
                                            EXP8_A, EXP8_B, ALU.mult, ALU.add)
                else:
                    nc.vector.tensor_scalar(dst.bitcast(I16), sps[:],
                                            EXP_A, EXP_B, ALU.mult, ALU.add)
            else:
                nc.scalar.activation(dst, sps[:], AF.Exp, scale=float(D) ** -0.5)

        def scores_exp(m, u, j, ps, f_dve, pairs):
            r, i = divmod(j, 2)
            sps = scores_mm(m, u, j, ps)
            if r in S_BF16:
                pt = work.tile([P, 2 * QB], BF16, name=f"{pfx}pt{m}{u}{j}", tag="pt")
                exp_into(sps, pt[:], f_dve, fp8=False)
                pairs.setdefault(r, []).append(pt)
            else:
                if i == 0:
                    pairs[r] = work.tile([P, 2, 2 * QB], F8,
                                         name=f"{pfx}pt8{m}{u}{r}", tag="pt")
                exp_into(sps, pairs[r][:, i, :], f_dve, fp8=True)

        def ctx_pair(m, u, r, pairs, cps):
            start, stop = (r == 0), (r == NT // 2 - 1)
            if r in S_BF16:
                for i, pt in enumerate(pairs[r]):
                    for g in range(2):
                        nc.tensor.matmul(cps[:, QB * g:QB * (g + 1)],
                                         va[m][2 * r + i][g][:],
                                         pt[:, QB * g:QB * (g + 1)],
                                         start=start and i == 0,
                                         stop=stop and i == 1)
            else:
                ptp = pairs[r]
                for g in range(2):
                    nc.tensor.matmul(cps[:, QB * g:QB * (g + 1)],
                                     va8[m][r][g][:, :, 0:D + 1],
                                     ptp[:, :, QB * g:QB * (g + 1)],
                                     start=start, stop=stop, perf_mode=DROW)

        def normalize_release(m, u, cps, fast=False):
            # part 1: free the cps bank + pull the denominators; emitted
            # BEFORE the next block's preloaded scores so the cu copies sit
            # ahead of fresh exps in the DVE/ACT queues.
            cu = norm.tile([D + 1, 2 * QB], F32, name=f"{pfx}cu{m}{u}", tag="cu")
            nc.vector.tensor_copy(cu[:, 0:QB], cps[:, 0:QB])
            nc.scalar.copy(cu[:, QB:2 * QB], cps[:, QB:2 * QB])
            s_sb = norm.tile([1, 2 * QB], F32, name=f"{pfx}ssb{m}{u}", tag="ssb")
            nc.vector.tensor_copy(s_sb[:], cu[D:D + 1, :])
            return cu, s_sb

        def normalize(m, u, rel, fast_ps=None):
            q0 = QB * u
            cu, s_sb = rel
            rr = norm.tile([1, 2 * QB], F32, name=f"{pfx}rr{m}{u}", tag="rr")
            nc.vector.reciprocal_approx_fast(rr[:], s_sb[:])
            if fast_ps is not None:
                # tail only: the PE is idle here, so a K=1 bf16 matmul beats
                # the ~2us gpsimd broadcast on the critical path to outproj
                rrb = norm.tile([1, 2 * QB], BF16, name=f"{pfx}rrb{m}{u}", tag="rrb")
                nc.vector.tensor_copy(rrb[:], rr[:])
                rb = fast_ps.tile([D, 2 * QB], F32, name=f"{pfx}rbp{m}{u}", tag="ps")
                for g in range(2):
                    nc.tensor.matmul(rb[:, QB * g:QB * (g + 1)], ones_b[:],
                                     rrb[:, QB * g:QB * (g + 1)],
                                     start=True, stop=True)
            else:
                rb = norm.tile([D, 2 * QB], F32, name=f"{pfx}rb{m}{u}", tag="rb")
                nc.gpsimd.partition_broadcast(rb[:], rr[:])
            nc.vector.tensor_mul(ctxT[m][0:D, q0:q0 + QB], cu[0:D, 0:QB], rb[:, 0:QB])
            nc.vector.tensor_mul(ctxT[m][D:P, q0:q0 + QB], cu[0:D, QB:2 * QB], rb[:, QB:2 * QB])

        def outproj_t(t_i, ps, ob_act=False):
            pso = ps.tile([P, C], F32, name=f"{pfx}pso{t_i}", tag="ps")
            for m in range(MC):
                nc.tensor.matmul(pso[:, 0:512], ctxT[m][:, P * t_i:P * (t_i + 1)],
                                 wo_b[m][:, 0:512], start=(m == 0), stop=(m == MC - 1))
                nc.tensor.matmul(pso[:, 512:C], ctxT[m][:, P * t_i:P * (t_i + 1)],
                                 wo_b[m][:, 512:C], start=(m == 0), stop=(m == MC - 1))
            ob = outp.tile([P, C], F32, name=f"{pfx}ob{t_i}", tag="ob")
            if ob_act:
                nc.scalar.copy(ob[:], pso[:])
            else:
                nc.vector.tensor_copy(ob[:], pso[:])
            # alternate hardware DMA queues so the tail's 4 stores overlap
            eng = nc.scalar if t_i % 2 else nc.sync
            eng.dma_start(out[P * t_i:P * (t_i + 1), :], ob[:])

        def attn_block(m, u, ps, cpsp, f_dve, projs=(), outs=(), pre=None,
                       nxt=None, f_nxt=None, fast_norm=False):
            # skew-2, 2-step grouped emission: [s_{j+2}, s_{j+3}] then
            # [c_j, c_{j+1}] — halves the PE 64<->128 tile-config switches
            # (an unhidden LDWEIGHTS, ~135ns) and keeps the exp engines 2-3
            # chunks ahead of the ctx consumer. Hooks (projections/outproj
            # bursts) land between a scores group and its ctx group, deep in
            # the block where the exp backlog can absorb the PE detour.
            # The last step pre-emits the NEXT block's first two scores so
            # the exp stream never drains at block boundaries.
            cps = cpsp.tile([D + 1, 2 * QB], F32, name=f"{pfx}cps{m}_{u}", tag="cps")
            hooks = {}
            for i, fn in enumerate(projs):
                hooks.setdefault({0: 4, 1: 8, 2: 12}[i], []).append(fn)
            for j_at, fn in outs:
                hooks.setdefault(j_at, []).append(fn)
            pairs = {}
            if pre is not None:
                pairs[0] = pre
            else:
                scores_exp(m, u, 0, ps, f_dve, pairs)
                scores_exp(m, u, 1, ps, f_dve, pairs)
            nxt_pair = None
            for j in range(0, NT, 2):
                if j + 2 < NT:
                    scores_exp(m, u, j + 2, ps, f_dve, pairs)
                    scores_exp(m, u, j + 3, ps, f_dve, pairs)
                elif nxt is not None:
                    npairs = {}
                    scores_exp(nxt[0], nxt[1], 0, ps, f_nxt, npairs)
                    scores_exp(nxt[0], nxt[1], 1, ps, f_nxt, npairs)
                    nxt_pair = npairs[0]
                for fn in hooks.get(j, ()):
                    fn()
                ctx_pair(m, u, j // 2, pairs, cps)
            normalize(m, u, normalize_release(m, u, cps, fast=fast_norm),
                      fast_ps=ps if fast_norm else None)
            return nxt_pair

        # ================= emission =================
        with tc.tile_pool(name=pfx + "xrp", bufs=4) as xrp, \
             tc.tile_pool(name=pfx + "xbp", bufs=5) as xbp, \
             tc.tile_pool(name=pfx + "wstage", bufs=3) as wstage, \
             tc.tile_pool(name=pfx + "ps", bufs=3, space="PSUM") as ps, \
             tc.tile_pool(name=pfx + "cps", bufs=1, space="PSUM") as cpsp:
            # weights first (k/q gate the first projections, v the v-chunks;
            # all casts clear the ACT queue before any exps land there)
            emit_weight_loads(wstage, [("k", wk), ("q", wq), ("v", wv)])
            emit_xt_group(0, xrp, xbp, ps)
            cps00 = cpsp.tile([D + 1, 2 * QB], F32, name=f"{pfx}cps0_0", tag="cps")
            pairs00 = {}
            emit_proj_block("k", 0, 0, ps)
            emit_proj_block("q", 0, 0, ps)
            for j in range(4):
                scores_exp(0, 0, j, ps, F_DVE_LEAD, pairs00)
            for tq in range(1, 4):
                emit_xt_group(tq, xrp, xbp, ps)
                emit_proj_block("k", 0, tq, ps)
                emit_proj_block("q", 0, tq, ps)
                for j in range(4 * tq, 4 * (tq + 1)):
                    scores_exp(0, 0, j, ps, F_DVE_LEAD, pairs00)
            for t_i in range(NT):
                emit_v_chunk(t_i, ps)
            emit_weight_loads(wstage, [("o", wo)])
            for r in range(NT // 2):
                ctx_pair(0, 0, r, pairs00, cps00)
            normalize(0, 0, normalize_release(0, 0, cps00))

            K = lambda m, n: (lambda: emit_proj_block("k", m, n, ps))
            Q = lambda m, n: (lambda: emit_proj_block("q", m, n, ps))
            OT = lambda t: (lambda: outproj_t(t, ps))
            # outproj groups run >=1 full block after their normalize so the
            # PE never waits on the recip/broadcast chain; the last block
            # absorbs groups 1 AND 2, leaving only group 3 for the tail.
            sched = [
                dict(b=(0, 1), f=F_DVE_MAIN),
                dict(b=(0, 2), f=F_DVE_MAIN, projs=[K(1, 0), Q(1, 0), K(1, 1)]),
                dict(b=(0, 3), f=F_DVE_MAIN, projs=[Q(1, 1), K(1, 2), Q(1, 2)]),
                dict(b=(1, 0), f=F_DVE_MAIN, projs=[K(1, 3), Q(1, 3)]),
                dict(b=(1, 1), f=F_DVE_MAIN, projs=[K(2, 0), Q(2, 0)]),
                dict(b=(1, 2), f=F_DVE_MAIN, projs=[K(2, 1), Q(2, 1)]),
                dict(b=(1, 3), f=F_DVE_MAIN, projs=[K(2, 2), Q(2, 2)]),
                dict(b=(2, 0), f=F_DVE_OUT, projs=[K(2, 3), Q(2, 3)]),
                dict(b=(2, 1), f=F_DVE_OUT),
                dict(b=(2, 2), f=F_DVE_OUT,
                     outs=[(4, OT(0)), (8, OT(1)), (12, OT(2)), (14, OT(3))]),
                dict(b=(2, 3), f=F_DVE_OUT, fast_norm=True,
                     outs=[(4, OT(4)), (6, OT(5)), (8, OT(6)), (10, OT(7)),
                           (12, OT(8)), (12, OT(9)), (14, OT(10)), (14, OT(11))]),
            ]
            pre = None
            for i, s in enumerate(sched):
                nxt = sched[i + 1] if i + 1 < len(sched) else None
                pre = attn_block(s["b"][0], s["b"][1], ps, cpsp, s["f"],
                                 projs=s.get("projs", ()), outs=s.get("outs", ()),
                                 pre=pre,
                                 nxt=nxt["b"] if nxt else None,
                                 f_nxt=nxt["f"] if nxt else None,
                                 fast_norm=s.get("fast_norm", False))
            for t_i in range(12, 16):
                outproj_t(t_i, ps, ob_act=(t_i % 2 == 0))

    with tile.TileContext(nc) as tc, ExitStack() as ctx:
        consts = ctx.enter_context(tc.tile_pool(name="consts", bufs=1))
        ident_f32 = consts.tile([P, P], mybir.dt.float32)
        make_identity(nc, ident_f32)
        ident_bf = consts.tile([P, P], BF16)
        nc.vector.tensor_copy(ident_bf[:], ident_f32[:])

        big = ctx.enter_context(tc.tile_pool(name="big", bufs=12))
        wrp = ctx.enter_context(tc.tile_pool(name="wrp", bufs=1))
        vap = ctx.enter_context(tc.tile_pool(name="vap", bufs=1))
        work = ctx.enter_context(tc.tile_pool(name="work", bufs=20))
        outp = ctx.enter_context(tc.tile_pool(name="outp", bufs=2))
        norm = ctx.enter_context(tc.tile_pool(name="norm", bufs=2))
        pools = (ident_bf, big, wrp, vap, work, outp, norm)
        for rep in range(repeat):
            emit(f"r{rep}_", tc, pools)

    nc.compile()
    return nc


def kernel(X, Wq, Wk, Wv, Wo, bo):
    from concourse import bass_utils

    if "nc" not in _cache:
        _cache["nc"] = _build(int(os.environ.get("KERNEL_REPEAT", "1")))
    nc = _cache["nc"]

    X = np.asarray(X, dtype=np.float32)
    in_maps = []
    for c in range(8):
        b, g = divmod(c, 2)
        sl = slice(HD * g, HD * (g + 1))
        in_maps.append({
            "x": np.ascontiguousarray(X[b]),
            "wq": np.ascontiguousarray(np.asarray(Wq, np.float32)[:, sl]),
            "wk": np.ascontiguousarray(np.asarray(Wk, np.float32)[:, sl]),
            "wv": np.ascontiguousarray(np.asarray(Wv, np.float32)[:, sl]),
            "wo": np.ascontiguousarray(np.asarray(Wo, np.float32)[sl, :]),
        })
    res = bass_utils.run_bass_kernel_spmd(nc, in_maps, core_ids=list(range(8)))
    _cache["last_result"] = res
    outf = np.empty((4, T, C), np.float32)
    bo = np.asarray(bo, np.float32)
    for b in range(4):
        outf[b] = res.results[2 * b]["out"] + res.results[2 * b + 1]["out"] + bo
    return outf


# revision 56
# speedup vs baseline: 1.0165x; 1.0165x over previous
"""Multi-head self-attention (B=4, T=2048, C=768, H=12) on 8 trn2 NeuronCores. v19.

Sharding: core c -> batch b=c//2, head-group g=c%2 (6 heads each).
Host sums the 2 partials per batch and adds the bias.

Key mechanisms (evolved v9 -> v19):
  - skew-2, 2-step grouped attention pipeline: [s_{j+2}, s_{j+3}] emitted
    before [c_j, c_{j+1}]; the in-order PE queue never head-of-line blocks
    on exp latency, and 64<->128 tile-config switches (unhidden LDWEIGHTS)
    are halved. The last step preloads the NEXT block's first score pair so
    the ACT/DVE exp streams never drain at block boundaries.
  - one shared PSUM pool (3 x [128,1024] f32 = 6 banks) serves scores,
    projections, v chunks, transposes and outproj; cps gets the other 2.
  - mixed-precision ctx: key-chunk pairs outside S_BF16 run exp->fp8e4
    (ACT exp or DVE Schraudolph-to-int8 bits) and a DoubleRow fp8 matmul
    (double contraction per pass); S_BF16 pairs keep bf16 exp + classic
    matmuls to hold max-rel error ~1.5e-2 < 2e-2. va for fp8 pairs is
    pair-interleaved [128,2,80] (DoubleRow needs 16B-aligned plane stride).
  - exp engine split ACT/DVE by fractional accumulator (~0.34 DVE).
  - x DMAs ride the ACT hardware DMA queue, weights the SP queue (parallel
    streams cut the serial-DMA lead-in); out stores alternate queues.
  - normalize: cu release split DVE/ACT frees the cps bank in ~600ns;
    recip + gpsimd partition-broadcast + muls run off the critical path;
    the last block broadcasts via a K=1 bf16 PE matmul instead (PE is idle
    in the tail) and outproj groups trail their normalize by >=1 block.
"""
import sys
import os

sys.path.insert(0, "/opt/trn_rl_repo")

import numpy as np

P = 128
T = 2048
C = 768
HD = 384          # per-core head columns (6 heads x 64)
D = 64
NT = T // P       # 16 key chunks of 128
KC = C // P       # 6 contraction chunks for C
MC = HD // P      # 3 head pairs
QB = 512          # query block
NQ = T // QB      # 4 query blocks

EXP_A = 128 * 0.125 * float(np.log2(np.e))   # 23.083120654223414
EXP_B = 128 * 127 - 5.5                      # 16250.5 (Schraudolph bias, bf16)
EXP8_A = 8 * 0.125 * float(np.log2(np.e))    # 1.4427 (Schraudolph, fp8e4m3)
EXP8_B = 8 * 7 - 0.46                        # 55.54

# key-chunk PAIRS (of 8) that keep bf16 exp + classic ctx matmuls; the rest
# run exp->fp8e4 and a DoubleRow ctx matmul (2x PE throughput). Limiting the
# fp8 fraction keeps the max-rel error ~1.4e-2 < 2e-2.
S_BF16 = frozenset({1, 4, 6})

# DVE share of exp chunks per phase (fractional accumulator)
F_DVE_LEAD = 0.34    # block (0,0): DVE busy with lead-in casts/copies
F_DVE_MAIN = 0.34    # blocks m=0 u>=1 .. m=1
F_DVE_OUT = 0.25     # m=2 phase: DVE also does outproj copies + norm muls

_cache = {}


def _build(repeat=1):
    import concourse.bacc as bacc
    import concourse.mybir as mybir
    import concourse.tile as tile
    from concourse.masks import make_identity
    from contextlib import ExitStack

    F32 = mybir.dt.float32
    BF16 = mybir.dt.bfloat16
    I16 = mybir.dt.int16
    F8 = mybir.dt.float8e4
    I8 = mybir.dt.int8
    AF = mybir.ActivationFunctionType
    ALU = mybir.AluOpType
    DROW = mybir.MatmulPerfMode.DoubleRow

    nc = bacc.Bacc("TRN2", target_bir_lowering=False, debug=False)
    x = nc.dram_tensor("x", [T, C], F32, kind="ExternalInput").ap()
    wq = nc.dram_tensor("wq", [C, HD], F32, kind="ExternalInput").ap()
    wk = nc.dram_tensor("wk", [C, HD], F32, kind="ExternalInput").ap()
    wv = nc.dram_tensor("wv", [C, HD], F32, kind="ExternalInput").ap()
    wo = nc.dram_tensor("wo", [HD, C], F32, kind="ExternalInput").ap()
    out = nc.dram_tensor("out", [T, C], F32, kind="ExternalOutput").ap()

    def emit(pfx, tc, pools):
        ident_bf, big, wrp, vap, work, outp, norm = pools

        xt = [big.tile([P, T], BF16, name=f"{pfx}xt{kc}", tag="big2048") for kc in range(KC)]
        qT = [big.tile([P, T], BF16, name=f"{pfx}qT{m}", tag="big2048") for m in range(MC)]
        kT = [big.tile([P, T], BF16, name=f"{pfx}kT{m}", tag="big2048") for m in range(MC)]
        ctxT = [big.tile([P, T], BF16, name=f"{pfx}ctxT{m}", tag="big2048") for m in range(MC)]
        # v tiles: bf16 pairs keep the classic [P, D+1] per-chunk layout;
        # fp8 pairs hold both chunks of the pair interleaved on the middle
        # dim, padded to 80 cols (DoubleRow needs a 16B-aligned plane stride)
        va = [[[(vap.tile([P, D + 1], BF16, name=f"{pfx}va{m}_{t}_{g}",
                          tag=f"va{m}_{t}_{g}") if (t // 2) in S_BF16 else None)
                for g in range(2)] for t in range(NT)] for m in range(MC)]
        va8 = [[[(None if r in S_BF16 else
                  vap.tile([P, 2, 80], F8, name=f"{pfx}va8{m}_{r}_{g}",
                           tag=f"va8{m}_{r}_{g}"))
                 for g in range(2)] for r in range(NT // 2)] for m in range(MC)]

        w_b = {}
        wo_b = []
        exp_acc = [0.0]
        ones1 = norm.tile([1, D], F32, name=f"{pfx}ones1", tag="ones1")
        nc.gpsimd.memset(ones1[:], 1.0)
        ones_b = norm.tile([1, D], BF16, name=f"{pfx}onesb", tag="onesb")
        nc.vector.tensor_copy(ones_b[:], ones1[:])

        def emit_weight_loads(wstage, names):
            for nm, src in names:
                if nm == "o":
                    for m in range(MC):
                        st = wstage.tile([P, C], F32, name=f"{pfx}wst_o{m}", tag="wsto")
                        nc.sync.dma_start(st[:], wo[P * m:P * (m + 1), :])
                        t_b = wrp.tile([P, C], BF16, name=f"{pfx}wo_{m}", tag=f"wo_{m}")
                        nc.gpsimd.tensor_copy(t_b[:], st[:])
                        wo_b.append(t_b)
                    continue
                for kc in range(KC):
                    st = wstage.tile([P, HD], F32, name=f"{pfx}wst_{nm}{kc}", tag="wst")
                    nc.sync.dma_start(st[:], src[P * kc:P * (kc + 1), :])
                    t_b = wrp.tile([P, HD], BF16, name=f"{pfx}w_{nm}{kc}", tag=f"w_{nm}{kc}")
                    # spread weight casts by criticality: 18 serial ACT
                    # copies (~11us) delayed both the first projections and
                    # the first exp chunks. k gates the first proj -> ACT;
                    # q -> DVE (ahead of most x casts); v -> idle gpsimd
                    # (slow DSP casts, but emitted up front they finish
                    # before the v-chunk phase needs them).
                    if nm == "k":
                        nc.scalar.copy(t_b[:], st[:])
                    elif nm == "q":
                        nc.vector.tensor_copy(t_b[:], st[:])
                    else:
                        nc.gpsimd.tensor_copy(t_b[:], st[:])
                    w_b[nm, kc] = t_b

        def emit_xt_group(tq, xrp, xbp, ps):
            xbs = []
            for i in range(4):
                t_i = 4 * tq + i
                xr = xrp.tile([P, C], F32, name=f"{pfx}xr{t_i}", tag="xr")
                # ACT hardware DMA queue: runs parallel to the SP queue that
                # carries the weight loads (serial DMAs cost ~10us lead-in)
                nc.scalar.dma_start(xr[:], x[P * t_i:P * (t_i + 1), :])
                xb = xbp.tile([P, C], BF16, name=f"{pfx}xb{t_i}", tag="xb")
                nc.vector.tensor_copy(xb[:], xr[:])
                xbs.append(xb)
            for kc in range(KC):
                tp = ps.tile([P, 512], BF16, name=f"{pfx}tp_{tq}_{kc}", tag="ps")
                for i in range(4):
                    nc.tensor.transpose(tp[:, P * i:P * (i + 1)],
                                        xbs[i][:, P * kc:P * (kc + 1)], ident_bf[:])
                nc.vector.tensor_copy(xt[kc][:, 512 * tq:512 * (tq + 1)], tp[:])

        def emit_proj_block(nm, m, n, ps):
            dest = qT if nm == "q" else kT
            pst = ps.tile([P, 512], F32, name=f"{pfx}ps_{nm}{m}{n}", tag="ps")
            for kc in range(KC):
                nc.tensor.matmul(
                    pst[:],
                    w_b[nm, kc][:, P * m:P * (m + 1)],
                    xt[kc][:, 512 * n:512 * (n + 1)],
                    start=(kc == 0), stop=(kc == KC - 1),
                )
            if m == 0:
                nc.scalar.copy(dest[m][:, 512 * n:512 * (n + 1)], pst[:])
            else:
                nc.vector.tensor_copy(dest[m][:, 512 * n:512 * (n + 1)], pst[:])

        def emit_v_chunk(t_i, ps):
            pv = ps.tile([P, HD], F32, name=f"{pfx}pv{t_i}", tag="ps")
            for kc in range(KC):
                nc.tensor.matmul(
                    pv[:],
                    xt[kc][:, P * t_i:P * (t_i + 1)],
                    w_b["v", kc][:],
                    start=(kc == 0), stop=(kc == KC - 1),
                )
            r, i = divmod(t_i, 2)
            for m in range(MC):
                for g in range(2):
                    src = pv[:, P * m + D * g:P * m + D * (g + 1)]
                    if r in S_BF16:
                        vt = va[m][t_i][g]
                        nc.vector.tensor_copy(vt[:, 0:D], src)
                        nc.gpsimd.memset(vt[:, D:D + 1], 1.0)
                    else:
                        vt = va8[m][r][g]
                        nc.vector.tensor_copy(vt[:, i, 0:D], src)
                        nc.gpsimd.memset(vt[:, i, D:D + 1], 1.0)

        def scores_mm(m, u, j, ps):
            q0 = QB * u
            sps = ps.tile([P, 2 * QB], F32, name=f"{pfx}sps{m}{u}{j}", tag="ps")
            nc.tensor.matmul(sps[:, 0:QB],
                             kT[m][0:D, P * j:P * (j + 1)],
                             qT[m][0:D, q0:q0 + QB],
                             start=True, stop=True, tile_position=(0, 0))
            nc.tensor.matmul(sps[:, QB:2 * QB],
                             kT[m][D:P, P * j:P * (j + 1)],
                             qT[m][D:P, q0:q0 + QB],
                             start=True, stop=True, tile_position=(64, 0))
            return sps

        def exp_into(sps, dst, f_dve, fp8):
            exp_acc[0] += f_dve
            if exp_acc[0] >= 1.0:
                exp_acc[0] -= 1.0
                if fp8:
                    nc.vector.tensor_scalar(dst.bitcast(I8), sps[:],
                                            EXP8_A, EXP8_B, ALU.mult, ALU.add)
                else:
                    nc.vector.tensor_scalar(dst.bitcast(I16), sps[:],
                                            EXP_A, EXP_B, ALU.mult, ALU.add)
            else:
                nc.scalar.activation(dst, sps[:], AF.Exp, scale=float(D) ** -0.5)

        def scores_exp(m, u, j, ps, f_dve, pairs):
            r, i = divmod(j, 2)
            sps = scores_mm(m, u, j, ps)
            if r in S_BF16:
                pt = work.tile([P, 2 * QB], BF16, name=f"{pfx}pt{m}{u}{j}", tag="pt")
                exp_into(sps, pt[:], f_dve, fp8=False)
                pairs.setdefault(r, []).append(pt)
            else:
                if i == 0:
                    pairs[r] = work.tile([P, 2, 2 * QB], F8,
                                         name=f"{pfx}pt8{m}{u}{r}", tag="pt")
                exp_into(sps, pairs[r][:, i, :], f_dve, fp8=True)

        def ctx_pair(m, u, r, pairs, cps):
            start, stop = (r == 0), (r == NT // 2 - 1)
            if r in S_BF16:
                for i, pt in enumerate(pairs[r]):
                    for g in range(2):
                        nc.tensor.matmul(cps[:, QB * g:QB * (g + 1)],
                                         va[m][2 * r + i][g][:],
                                         pt[:, QB * g:QB * (g + 1)],
                                         start=start and i == 0,
                                         stop=stop and i == 1)
            else:
                ptp = pairs[r]
                for g in range(2):
                    nc.tensor.matmul(cps[:, QB * g:QB * (g + 1)],
                                     va8[m][r][g][:, :, 0:D + 1],
                                     ptp[:, :, QB * g:QB * (g + 1)],
                                     start=start, stop=stop, perf_mode=DROW)

        def normalize_release(m, u, cps, fast=False):
            # part 1: free the cps bank + pull the denominators; emitted
            # BEFORE the next block's preloaded scores so the cu copies sit
            # ahead of fresh exps in the DVE/ACT queues.
            cu = norm.tile([D + 1, 2 * QB], F32, name=f"{pfx}cu{m}{u}", tag="cu")
            nc.vector.tensor_copy(cu[:, 0:QB], cps[:, 0:QB])
            nc.scalar.copy(cu[:, QB:2 * QB], cps[:, QB:2 * QB])
            s_sb = norm.tile([1, 2 * QB], F32, name=f"{pfx}ssb{m}{u}", tag="ssb")
            nc.vector.tensor_copy(s_sb[:], cu[D:D + 1, :])
            return cu, s_sb

        def normalize(m, u, rel, fast_ps=None):
            q0 = QB * u
            cu, s_sb = rel
            rr = norm.tile([1, 2 * QB], F32, name=f"{pfx}rr{m}{u}", tag="rr")
            nc.vector.reciprocal_approx_fast(rr[:], s_sb[:])
            if fast_ps is not None:
                # tail only: the PE is idle here, so a K=1 bf16 matmul beats
                # the ~2us gpsimd broadcast on the critical path to outproj
                rrb = norm.tile([1, 2 * QB], BF16, name=f"{pfx}rrb{m}{u}", tag="rrb")
                nc.vector.tensor_copy(rrb[:], rr[:])
                rb = fast_ps.tile([D, 2 * QB], F32, name=f"{pfx}rbp{m}{u}", tag="ps")
                for g in range(2):
                    nc.tensor.matmul(rb[:, QB * g:QB * (g + 1)], ones_b[:],
                                     rrb[:, QB * g:QB * (g + 1)],
                                     start=True, stop=True)
            else:
                rb = norm.tile([D, 2 * QB], F32, name=f"{pfx}rb{m}{u}", tag="rb")
                nc.gpsimd.partition_broadcast(rb[:], rr[:])
            nc.vector.tensor_mul(ctxT[m][0:D, q0:q0 + QB], cu[0:D, 0:QB], rb[:, 0:QB])
            nc.vector.tensor_mul(ctxT[m][D:P, q0:q0 + QB], cu[0:D, QB:2 * QB], rb[:, QB:2 * QB])

        def outproj_t(t_i, ps, ob_act=False):
            pso = ps.tile([P, C], F32, name=f"{pfx}pso{t_i}", tag="ps")
            for m in range(MC):
                nc.tensor.matmul(pso[:, 0:512], ctxT[m][:, P * t_i:P * (t_i + 1)],
                                 wo_b[m][:, 0:512], start=(m == 0), stop=(m == MC - 1))
                nc.tensor.matmul(pso[:, 512:C], ctxT[m][:, P * t_i:P * (t_i + 1)],
                                 wo_b[m][:, 512:C], start=(m == 0), stop=(m == MC - 1))
            ob = outp.tile([P, C], F32, name=f"{pfx}ob{t_i}", tag="ob")
            if ob_act:
                nc.scalar.copy(ob[:], pso[:])
            else:
                nc.vector.tensor_copy(ob[:], pso[:])
            # alternate hardware DMA queues so the tail's 4 stores overlap
            eng = nc.scalar if t_i % 2 else nc.sync
            eng.dma_start(out[P * t_i:P * (t_i + 1), :], ob[:])

        def attn_block(m, u, ps, cpsp, f_dve, projs=(), outs=(), pre=None,
                       nxt=None, f_nxt=None, fast_norm=False):
            # skew-2, 2-step grouped emission: [s_{j+2}, s_{j+3}] then
            # [c_j, c_{j+1}] — halves the PE 64<->128 tile-config switches
            # (an unhidden LDWEIGHTS, ~135ns) and keeps the exp engines 2-3
            # chunks ahead of the ctx consumer. Hooks (projections/outproj
            # bursts) land between a scores group and its ctx group, deep in
            # the block where the exp backlog can absorb the PE detour.
            # The last step pre-emits the NEXT block's first two scores so
            # the exp stream never drains at block boundaries.
            cps = cpsp.tile([D + 1, 2 * QB], F32, name=f"{pfx}cps{m}_{u}", tag="cps")
            hooks = {}
            for i, fn in enumerate(projs):
                hooks.setdefault({0: 4, 1: 8, 2: 12}[i], []).append(fn)
            for j_at, fn in outs:
                hooks.setdefault(j_at, []).append(fn)
            pairs = {}
            if pre is not None:
                pairs[0] = pre
            else:
                scores_exp(m, u, 0, ps, f_dve, pairs)
                scores_exp(m, u, 1, ps, f_dve, pairs)
            nxt_pair = None
            for j in range(0, NT, 2):
                if j + 2 < NT:
                    scores_exp(m, u, j + 2, ps, f_dve, pairs)
                    scores_exp(m, u, j + 3, ps, f_dve, pairs)
                elif nxt is not None:
                    npairs = {}
                    scores_exp(nxt[0], nxt[1], 0, ps, f_nxt, npairs)
                    scores_exp(nxt[0], nxt[1], 1, ps, f_nxt, npairs)
                    nxt_pair = npairs[0]
                for fn in hooks.get(j, ()):
                    fn()
                ctx_pair(m, u, j // 2, pairs, cps)
            normalize(m, u, normalize_release(m, u, cps, fast=fast_norm),
                      fast_ps=ps if fast_norm else None)
            return nxt_pair

        # ================= emission =================
        with tc.tile_pool(name=pfx + "xrp", bufs=4) as xrp, \
             tc.tile_pool(name=pfx + "xbp", bufs=5) as xbp, \
             tc.tile_pool(name=pfx + "wstage", bufs=3) as wstage, \
             tc.tile_pool(name=pfx + "ps", bufs=3, space="PSUM") as ps, \
             tc.tile_pool(name=pfx + "cps", bufs=1, space="PSUM") as cpsp:
            # k first (gates the first proj); x group 0 next so its DVE
            # casts precede the q casts in the DVE queue; v casts trail on
            # gpsimd, done well before the v-chunk phase
            emit_weight_loads(wstage, [("k", wk)])
            emit_xt_group(0, xrp, xbp, ps)
            emit_weight_loads(wstage, [("q", wq), ("v", wv)])
            cps00 = cpsp.tile([D + 1, 2 * QB], F32, name=f"{pfx}cps0_0", tag="cps")
            pairs00 = {}
            emit_proj_block("k", 0, 0, ps)
            emit_proj_block("q", 0, 0, ps)
            for j in range(4):
                scores_exp(0, 0, j, ps, F_DVE_LEAD, pairs00)
            for tq in range(1, 4):
                emit_xt_group(tq, xrp, xbp, ps)
                emit_proj_block("k", 0, tq, ps)
                emit_proj_block("q", 0, tq, ps)
                for j in range(4 * tq, 4 * (tq + 1)):
                    scores_exp(0, 0, j, ps, F_DVE_LEAD, pairs00)
            for t_i in range(NT):
                emit_v_chunk(t_i, ps)
            emit_weight_loads(wstage, [("o", wo)])
            for r in range(NT // 2):
                ctx_pair(0, 0, r, pairs00, cps00)
            normalize(0, 0, normalize_release(0, 0, cps00))

            K = lambda m, n: (lambda: emit_proj_block("k", m, n, ps))
            Q = lambda m, n: (lambda: emit_proj_block("q", m, n, ps))
            OT = lambda t: (lambda: outproj_t(t, ps))
            # outproj groups run >=1 full block after their normalize so the
            # PE never waits on the recip/broadcast chain; the last block
            # absorbs groups 1 AND 2, leaving only group 3 for the tail.
            sched = [
                dict(b=(0, 1), f=F_DVE_MAIN),
                dict(b=(0, 2), f=F_DVE_MAIN, projs=[K(1, 0), Q(1, 0), K(1, 1)]),
                dict(b=(0, 3), f=F_DVE_MAIN, projs=[Q(1, 1), K(1, 2), Q(1, 2)]),
                dict(b=(1, 0), f=F_DVE_MAIN, projs=[K(1, 3), Q(1, 3)]),
                dict(b=(1, 1), f=F_DVE_MAIN, projs=[K(2, 0), Q(2, 0)]),
                dict(b=(1, 2), f=F_DVE_MAIN, projs=[K(2, 1), Q(2, 1)]),
                dict(b=(1, 3), f=F_DVE_MAIN, projs=[K(2, 2), Q(2, 2)]),
                dict(b=(2, 0), f=F_DVE_OUT, projs=[K(2, 3), Q(2, 3)]),
                dict(b=(2, 1), f=F_DVE_OUT),
                dict(b=(2, 2), f=F_DVE_OUT,
                     outs=[(4, OT(0)), (8, OT(1)), (12, OT(2)), (14, OT(3))]),
                dict(b=(2, 3), f=F_DVE_OUT, fast_norm=True,
                     outs=[(4, OT(4)), (6, OT(5)), (8, OT(6)), (10, OT(7)),
                           (12, OT(8)), (12, OT(9)), (14, OT(10)), (14, OT(11))]),
            ]
            pre = None
            for i, s in enumerate(sched):
                nxt = sched[i + 1] if i + 1 < len(sched) else None
                pre = attn_block(s["b"][0], s["b"][1], ps, cpsp, s["f"],
                                 projs=s.get("projs", ()), outs=s.get("outs", ()),
                                 pre=pre,
                                 nxt=nxt["b"] if nxt else None,
                                 f_nxt=nxt["f"] if nxt else None,
                                 fast_norm=s.get("fast_norm", False))
            for t_i in range(12, 16):
                outproj_t(t_i, ps, ob_act=(t_i % 2 == 0))

    with tile.TileContext(nc) as tc, ExitStack() as ctx:
        consts = ctx.enter_context(tc.tile_pool(name="consts", bufs=1))
        ident_f32 = consts.tile([P, P], mybir.dt.float32)
        make_identity(nc, ident_f32)
        ident_bf = consts.tile([P, P], BF16)
        nc.vector.tensor_copy(ident_bf[:], ident_f32[:])

        big = ctx.enter_context(tc.tile_pool(name="big", bufs=12))
        wrp = ctx.enter_context(tc.tile_pool(name="wrp", bufs=1))
        vap = ctx.enter_context(tc.tile_pool(name="vap", bufs=1))
        work = ctx.enter_context(tc.tile_pool(name="work", bufs=20))
        outp = ctx.enter_context(tc.tile_pool(name="outp", bufs=2))
        norm = ctx.enter_context(tc.tile_pool(name="norm", bufs=2))
        pools = (ident_bf, big, wrp, vap, work, outp, norm)
        for rep in range(repeat):
            emit(f"r{rep}_", tc, pools)

    nc.compile()
    return nc


def kernel(X, Wq, Wk, Wv, Wo, bo):
    from concourse import bass_utils

    if "nc" not in _cache:
        _cache["nc"] = _build(int(os.environ.get("KERNEL_REPEAT", "1")))
    nc = _cache["nc"]

    X = np.asarray(X, dtype=np.float32)
    in_maps = []
    for c in range(8):
        b, g = divmod(c, 2)
        sl = slice(HD * g, HD * (g + 1))
        in_maps.append({
            "x": np.ascontiguousarray(X[b]),
            "wq": np.ascontiguousarray(np.asarray(Wq, np.float32)[:, sl]),
            "wk": np.ascontiguousarray(np.asarray(Wk, np.float32)[:, sl]),
            "wv": np.ascontiguousarray(np.asarray(Wv, np.float32)[:, sl]),
            "wo": np.ascontiguousarray(np.asarray(Wo, np.float32)[sl, :]),
        })
    res = bass_utils.run_bass_kernel_spmd(nc, in_maps, core_ids=list(range(8)))
    _cache["last_result"] = res
    outf = np.empty((4, T, C), np.float32)
    bo = np.asarray(bo, np.float32)
    for b in range(4):
        outf[b] = res.results[2 * b]["out"] + res.results[2 * b + 1]["out"] + bo
    return outf


# revision 57
# speedup vs baseline: 1.0171x; 1.0006x over previous
"""Multi-head self-attention (B=4, T=2048, C=768, H=12) on 8 trn2 NeuronCores. v19.

Sharding: core c -> batch b=c//2, head-group g=c%2 (6 heads each).
Host sums the 2 partials per batch and adds the bias.

Key mechanisms (evolved v9 -> v19):
  - skew-2, 2-step grouped attention pipeline: [s_{j+2}, s_{j+3}] emitted
    before [c_j, c_{j+1}]; the in-order PE queue never head-of-line blocks
    on exp latency, and 64<->128 tile-config switches (unhidden LDWEIGHTS)
    are halved. The last step preloads the NEXT block's first score pair so
    the ACT/DVE exp streams never drain at block boundaries.
  - one shared PSUM pool (3 x [128,1024] f32 = 6 banks) serves scores,
    projections, v chunks, transposes and outproj; cps gets the other 2.
  - mixed-precision ctx: key-chunk pairs outside S_BF16 run exp->fp8e4
    (ACT exp or DVE Schraudolph-to-int8 bits) and a DoubleRow fp8 matmul
    (double contraction per pass); S_BF16 pairs keep bf16 exp + classic
    matmuls to hold max-rel error ~1.5e-2 < 2e-2. va for fp8 pairs is
    pair-interleaved [128,2,80] (DoubleRow needs 16B-aligned plane stride).
  - exp engine split ACT/DVE by fractional accumulator (~0.34 DVE).
  - x DMAs ride the ACT hardware DMA queue, weights the SP queue (parallel
    streams cut the serial-DMA lead-in); out stores alternate queues.
  - normalize: cu release split DVE/ACT frees the cps bank in ~600ns;
    recip + gpsimd partition-broadcast + muls run off the critical path;
    the last block broadcasts via a K=1 bf16 PE matmul instead (PE is idle
    in the tail) and outproj groups trail their normalize by >=1 block.
"""
import sys
import os

sys.path.insert(0, "/opt/trn_rl_repo")

import numpy as np

P = 128
T = 2048
C = 768
HD = 384          # per-core head columns (6 heads x 64)
D = 64
NT = T // P       # 16 key chunks of 128
KC = C // P       # 6 contraction chunks for C
MC = HD // P      # 3 head pairs
QB = 512          # query block
NQ = T // QB      # 4 query blocks

EXP_A = 128 * 0.125 * float(np.log2(np.e))   # 23.083120654223414
EXP_B = 128 * 127 - 5.5                      # 16250.5 (Schraudolph bias, bf16)
EXP8_A = 8 * 0.125 * float(np.log2(np.e))    # 1.4427 (Schraudolph, fp8e4m3)
EXP8_B = 8 * 7 - 0.46                        # 55.54

# key-chunk PAIRS (of 8) that keep bf16 exp + classic ctx matmuls; the rest
# run exp->fp8e4 and a DoubleRow ctx matmul (2x PE throughput). Limiting the
# fp8 fraction keeps the max-rel error ~1.4e-2 < 2e-2.
S_BF16 = frozenset({1, 4, 6})

# DVE share of exp chunks per phase (fractional accumulator)
F_DVE_LEAD = 0.34    # block (0,0): DVE busy with lead-in casts/copies
F_DVE_MAIN = 0.34    # blocks m=0 u>=1 .. m=1
F_DVE_OUT = 0.25     # m=2 phase: DVE also does outproj copies + norm muls

_cache = {}


def _build(repeat=1):
    import concourse.bacc as bacc
    import concourse.mybir as mybir
    import concourse.tile as tile
    from concourse.masks import make_identity
    from contextlib import ExitStack

    F32 = mybir.dt.float32
    BF16 = mybir.dt.bfloat16
    I16 = mybir.dt.int16
    F8 = mybir.dt.float8e4
    I8 = mybir.dt.int8
    AF = mybir.ActivationFunctionType
    ALU = mybir.AluOpType
    DROW = mybir.MatmulPerfMode.DoubleRow

    nc = bacc.Bacc("TRN2", target_bir_lowering=False, debug=False)
    x = nc.dram_tensor("x", [T, C], F32, kind="ExternalInput").ap()
    wq = nc.dram_tensor("wq", [C, HD], F32, kind="ExternalInput").ap()
    wk = nc.dram_tensor("wk", [C, HD], F32, kind="ExternalInput").ap()
    wv = nc.dram_tensor("wv", [C, HD], F32, kind="ExternalInput").ap()
    wo = nc.dram_tensor("wo", [HD, C], F32, kind="ExternalInput").ap()
    out = nc.dram_tensor("out", [T, C], F32, kind="ExternalOutput").ap()

    def emit(pfx, tc, pools):
        ident_bf, big, wrp, vap, work, outp, norm = pools

        xt = [big.tile([P, T], BF16, name=f"{pfx}xt{kc}", tag="big2048") for kc in range(KC)]
        qT = [big.tile([P, T], BF16, name=f"{pfx}qT{m}", tag="big2048") for m in range(MC)]
        kT = [big.tile([P, T], BF16, name=f"{pfx}kT{m}", tag="big2048") for m in range(MC)]
        ctxT = [big.tile([P, T], BF16, name=f"{pfx}ctxT{m}", tag="big2048") for m in range(MC)]
        # v tiles: bf16 pairs keep the classic [P, D+1] per-chunk layout;
        # fp8 pairs hold both chunks of the pair interleaved on the middle
        # dim, padded to 80 cols (DoubleRow needs a 16B-aligned plane stride)
        va = [[[(vap.tile([P, D + 1], BF16, name=f"{pfx}va{m}_{t}_{g}",
                          tag=f"va{m}_{t}_{g}") if (t // 2) in S_BF16 else None)
                for g in range(2)] for t in range(NT)] for m in range(MC)]
        va8 = [[[(None if r in S_BF16 else
                  vap.tile([P, 2, 80], F8, name=f"{pfx}va8{m}_{r}_{g}",
                           tag=f"va8{m}_{r}_{g}"))
                 for g in range(2)] for r in range(NT // 2)] for m in range(MC)]

        w_b = {}
        wo_b = []
        exp_acc = [0.0]
        ones1 = norm.tile([1, D], F32, name=f"{pfx}ones1", tag="ones1")
        nc.gpsimd.memset(ones1[:], 1.0)
        ones_b = norm.tile([1, D], BF16, name=f"{pfx}onesb", tag="onesb")
        nc.vector.tensor_copy(ones_b[:], ones1[:])

        def emit_weight_loads(wstage, names):
            for nm, src in names:
                if nm == "o":
                    for m in range(MC):
                        st = wstage.tile([P, C], F32, name=f"{pfx}wst_o{m}", tag="wsto")
                        nc.sync.dma_start(st[:], wo[P * m:P * (m + 1), :])
                        t_b = wrp.tile([P, C], BF16, name=f"{pfx}wo_{m}", tag=f"wo_{m}")
                        nc.gpsimd.tensor_copy(t_b[:], st[:])
                        wo_b.append(t_b)
                    continue
                for kc in range(KC):
                    st = wstage.tile([P, HD], F32, name=f"{pfx}wst_{nm}{kc}", tag="wst")
                    nc.sync.dma_start(st[:], src[P * kc:P * (kc + 1), :])
                    t_b = wrp.tile([P, HD], BF16, name=f"{pfx}w_{nm}{kc}", tag=f"w_{nm}{kc}")
                    # spread weight casts by criticality: 18 serial ACT
                    # copies (~11us) delayed both the first projections and
                    # the first exp chunks. k gates the first proj -> ACT;
                    # q -> DVE (ahead of most x casts); v -> idle gpsimd
                    # (slow DSP casts, but emitted up front they finish
                    # before the v-chunk phase needs them).
                    if nm in ("k", "q"):
                        # both on ACT: q casts on DVE stalled the group-2/3
                        # x casts (in-order queue) behind the wq DMA
                        nc.scalar.copy(t_b[:], st[:])
                    else:
                        nc.gpsimd.tensor_copy(t_b[:], st[:])
                    w_b[nm, kc] = t_b

        def emit_xt_group(tq, xrp, xbp, ps):
            xbs = []
            for i in range(4):
                t_i = 4 * tq + i
                xr = xrp.tile([P, C], F32, name=f"{pfx}xr{t_i}", tag="xr")
                # ACT hardware DMA queue: runs parallel to the SP queue that
                # carries the weight loads (serial DMAs cost ~10us lead-in)
                nc.scalar.dma_start(xr[:], x[P * t_i:P * (t_i + 1), :])
                xb = xbp.tile([P, C], BF16, name=f"{pfx}xb{t_i}", tag="xb")
                nc.vector.tensor_copy(xb[:], xr[:])
                xbs.append(xb)
            for kc in range(KC):
                tp = ps.tile([P, 512], BF16, name=f"{pfx}tp_{tq}_{kc}", tag="ps")
                for i in range(4):
                    nc.tensor.transpose(tp[:, P * i:P * (i + 1)],
                                        xbs[i][:, P * kc:P * (kc + 1)], ident_bf[:])
                nc.vector.tensor_copy(xt[kc][:, 512 * tq:512 * (tq + 1)], tp[:])

        def emit_proj_block(nm, m, n, ps):
            dest = qT if nm == "q" else kT
            pst = ps.tile([P, 512], F32, name=f"{pfx}ps_{nm}{m}{n}", tag="ps")
            for kc in range(KC):
                nc.tensor.matmul(
                    pst[:],
                    w_b[nm, kc][:, P * m:P * (m + 1)],
                    xt[kc][:, 512 * n:512 * (n + 1)],
                    start=(kc == 0), stop=(kc == KC - 1),
                )
            if m == 0:
                nc.scalar.copy(dest[m][:, 512 * n:512 * (n + 1)], pst[:])
            else:
                nc.vector.tensor_copy(dest[m][:, 512 * n:512 * (n + 1)], pst[:])

        def emit_v_chunk(t_i, ps):
            pv = ps.tile([P, HD], F32, name=f"{pfx}pv{t_i}", tag="ps")
            for kc in range(KC):
                nc.tensor.matmul(
                    pv[:],
                    xt[kc][:, P * t_i:P * (t_i + 1)],
                    w_b["v", kc][:],
                    start=(kc == 0), stop=(kc == KC - 1),
                )
            r, i = divmod(t_i, 2)
            for m in range(MC):
                for g in range(2):
                    src = pv[:, P * m + D * g:P * m + D * (g + 1)]
                    if r in S_BF16:
                        vt = va[m][t_i][g]
                        nc.vector.tensor_copy(vt[:, 0:D], src)
                        nc.gpsimd.memset(vt[:, D:D + 1], 1.0)
                    else:
                        vt = va8[m][r][g]
                        nc.vector.tensor_copy(vt[:, i, 0:D], src)
                        nc.gpsimd.memset(vt[:, i, D:D + 1], 1.0)

        def scores_mm(m, u, j, ps):
            q0 = QB * u
            sps = ps.tile([P, 2 * QB], F32, name=f"{pfx}sps{m}{u}{j}", tag="ps")
            nc.tensor.matmul(sps[:, 0:QB],
                             kT[m][0:D, P * j:P * (j + 1)],
                             qT[m][0:D, q0:q0 + QB],
                             start=True, stop=True, tile_position=(0, 0))
            nc.tensor.matmul(sps[:, QB:2 * QB],
                             kT[m][D:P, P * j:P * (j + 1)],
                             qT[m][D:P, q0:q0 + QB],
                             start=True, stop=True, tile_position=(64, 0))
            return sps

        def exp_into(sps, dst, f_dve, fp8):
            exp_acc[0] += f_dve
            if exp_acc[0] >= 1.0:
                exp_acc[0] -= 1.0
                if fp8:
                    nc.vector.tensor_scalar(dst.bitcast(I8), sps[:],
                                            EXP8_A, EXP8_B, ALU.mult, ALU.add)
                else:
                    nc.vector.tensor_scalar(dst.bitcast(I16), sps[:],
                                            EXP_A, EXP_B, ALU.mult, ALU.add)
            else:
                nc.scalar.activation(dst, sps[:], AF.Exp, scale=float(D) ** -0.5)

        def scores_exp(m, u, j, ps, f_dve, pairs):
            r, i = divmod(j, 2)
            sps = scores_mm(m, u, j, ps)
            if r in S_BF16:
                pt = work.tile([P, 2 * QB], BF16, name=f"{pfx}pt{m}{u}{j}", tag="pt")
                exp_into(sps, pt[:], f_dve, fp8=False)
                pairs.setdefault(r, []).append(pt)
            else:
                if i == 0:
                    pairs[r] = work.tile([P, 2, 2 * QB], F8,
                                         name=f"{pfx}pt8{m}{u}{r}", tag="pt")
                exp_into(sps, pairs[r][:, i, :], f_dve, fp8=True)

        def ctx_pair(m, u, r, pairs, cps):
            start, stop = (r == 0), (r == NT // 2 - 1)
            if r in S_BF16:
                for i, pt in enumerate(pairs[r]):
                    for g in range(2):
                        nc.tensor.matmul(cps[:, QB * g:QB * (g + 1)],
                                         va[m][2 * r + i][g][:],
                                         pt[:, QB * g:QB * (g + 1)],
                                         start=start and i == 0,
                                         stop=stop and i == 1)
            else:
                ptp = pairs[r]
                for g in range(2):
                    nc.tensor.matmul(cps[:, QB * g:QB * (g + 1)],
                                     va8[m][r][g][:, :, 0:D + 1],
                                     ptp[:, :, QB * g:QB * (g + 1)],
                                     start=start, stop=stop, perf_mode=DROW)

        def normalize_release(m, u, cps, fast=False):
            # part 1: free the cps bank + pull the denominators; emitted
            # BEFORE the next block's preloaded scores so the cu copies sit
            # ahead of fresh exps in the DVE/ACT queues.
            cu = norm.tile([D + 1, 2 * QB], F32, name=f"{pfx}cu{m}{u}", tag="cu")
            nc.vector.tensor_copy(cu[:, 0:QB], cps[:, 0:QB])
            nc.scalar.copy(cu[:, QB:2 * QB], cps[:, QB:2 * QB])
            s_sb = norm.tile([1, 2 * QB], F32, name=f"{pfx}ssb{m}{u}", tag="ssb")
            nc.vector.tensor_copy(s_sb[:], cu[D:D + 1, :])
            return cu, s_sb

        def normalize(m, u, rel, fast_ps=None):
            q0 = QB * u
            cu, s_sb = rel
            rr = norm.tile([1, 2 * QB], F32, name=f"{pfx}rr{m}{u}", tag="rr")
            nc.vector.reciprocal_approx_fast(rr[:], s_sb[:])
            if fast_ps is not None:
                # tail only: the PE is idle here, so a K=1 bf16 matmul beats
                # the ~2us gpsimd broadcast on the critical path to outproj
                rrb = norm.tile([1, 2 * QB], BF16, name=f"{pfx}rrb{m}{u}", tag="rrb")
                nc.vector.tensor_copy(rrb[:], rr[:])
                rb = fast_ps.tile([D, 2 * QB], F32, name=f"{pfx}rbp{m}{u}", tag="ps")
                for g in range(2):
                    nc.tensor.matmul(rb[:, QB * g:QB * (g + 1)], ones_b[:],
                                     rrb[:, QB * g:QB * (g + 1)],
                                     start=True, stop=True)
            else:
                rb = norm.tile([D, 2 * QB], F32, name=f"{pfx}rb{m}{u}", tag="rb")
                nc.gpsimd.partition_broadcast(rb[:], rr[:])
            nc.vector.tensor_mul(ctxT[m][0:D, q0:q0 + QB], cu[0:D, 0:QB], rb[:, 0:QB])
            nc.vector.tensor_mul(ctxT[m][D:P, q0:q0 + QB], cu[0:D, QB:2 * QB], rb[:, QB:2 * QB])

        def outproj_t(t_i, ps, ob_act=False):
            pso = ps.tile([P, C], F32, name=f"{pfx}pso{t_i}", tag="ps")
            for m in range(MC):
                nc.tensor.matmul(pso[:, 0:512], ctxT[m][:, P * t_i:P * (t_i + 1)],
                                 wo_b[m][:, 0:512], start=(m == 0), stop=(m == MC - 1))
                nc.tensor.matmul(pso[:, 512:C], ctxT[m][:, P * t_i:P * (t_i + 1)],
                                 wo_b[m][:, 512:C], start=(m == 0), stop=(m == MC - 1))
            ob = outp.tile([P, C], F32, name=f"{pfx}ob{t_i}", tag="ob")
            if ob_act:
                nc.scalar.copy(ob[:], pso[:])
            else:
                nc.vector.tensor_copy(ob[:], pso[:])
            # alternate hardware DMA queues so the tail's 4 stores overlap
            eng = nc.scalar if t_i % 2 else nc.sync
            eng.dma_start(out[P * t_i:P * (t_i + 1), :], ob[:])

        def attn_block(m, u, ps, cpsp, f_dve, projs=(), outs=(), pre=None,
                       nxt=None, f_nxt=None, fast_norm=False):
            # skew-2, 2-step grouped emission: [s_{j+2}, s_{j+3}] then
            # [c_j, c_{j+1}] — halves the PE 64<->128 tile-config switches
            # (an unhidden LDWEIGHTS, ~135ns) and keeps the exp engines 2-3
            # chunks ahead of the ctx consumer. Hooks (projections/outproj
            # bursts) land between a scores group and its ctx group, deep in
            # the block where the exp backlog can absorb the PE detour.
            # The last step pre-emits the NEXT block's first two scores so
            # the exp stream never drains at block boundaries.
            cps = cpsp.tile([D + 1, 2 * QB], F32, name=f"{pfx}cps{m}_{u}", tag="cps")
            hooks = {}
            for i, fn in enumerate(projs):
                hooks.setdefault({0: 4, 1: 8, 2: 12}[i], []).append(fn)
            for j_at, fn in outs:
                hooks.setdefault(j_at, []).append(fn)
            pairs = {}
            if pre is not None:
                pairs[0] = pre
            else:
                scores_exp(m, u, 0, ps, f_dve, pairs)
                scores_exp(m, u, 1, ps, f_dve, pairs)
            nxt_pair = None
            for j in range(0, NT, 2):
                if j + 2 < NT:
                    scores_exp(m, u, j + 2, ps, f_dve, pairs)
                    scores_exp(m, u, j + 3, ps, f_dve, pairs)
                elif nxt is not None:
                    npairs = {}
                    scores_exp(nxt[0], nxt[1], 0, ps, f_nxt, npairs)
                    scores_exp(nxt[0], nxt[1], 1, ps, f_nxt, npairs)
                    nxt_pair = npairs[0]
                for fn in hooks.get(j, ()):
                    fn()
                ctx_pair(m, u, j // 2, pairs, cps)
            normalize(m, u, normalize_release(m, u, cps, fast=fast_norm),
                      fast_ps=ps if fast_norm else None)
            return nxt_pair

        # ================= emission =================
        with tc.tile_pool(name=pfx + "xrp", bufs=4) as xrp, \
             tc.tile_pool(name=pfx + "xbp", bufs=5) as xbp, \
             tc.tile_pool(name=pfx + "wstage", bufs=3) as wstage, \
             tc.tile_pool(name=pfx + "ps", bufs=3, space="PSUM") as ps, \
             tc.tile_pool(name=pfx + "cps", bufs=1, space="PSUM") as cpsp:
            # k first (gates the first proj); x group 0 next so its DVE
            # casts precede the q casts in the DVE queue; v casts trail on
            # gpsimd, done well before the v-chunk phase
            emit_weight_loads(wstage, [("k", wk)])
            emit_xt_group(0, xrp, xbp, ps)
            emit_weight_loads(wstage, [("q", wq), ("v", wv)])
            cps00 = cpsp.tile([D + 1, 2 * QB], F32, name=f"{pfx}cps0_0", tag="cps")
            pairs00 = {}
            emit_proj_block("k", 0, 0, ps)
            emit_proj_block("q", 0, 0, ps)
            for j in range(4):
                scores_exp(0, 0, j, ps, F_DVE_LEAD, pairs00)
            for tq in range(1, 4):
                emit_xt_group(tq, xrp, xbp, ps)
                emit_proj_block("k", 0, tq, ps)
                emit_proj_block("q", 0, tq, ps)
                for j in range(4 * tq, 4 * (tq + 1)):
                    scores_exp(0, 0, j, ps, F_DVE_LEAD, pairs00)
            for t_i in range(NT):
                emit_v_chunk(t_i, ps)
            emit_weight_loads(wstage, [("o", wo)])
            for r in range(NT // 2):
                ctx_pair(0, 0, r, pairs00, cps00)
            normalize(0, 0, normalize_release(0, 0, cps00))

            K = lambda m, n: (lambda: emit_proj_block("k", m, n, ps))
            Q = lambda m, n: (lambda: emit_proj_block("q", m, n, ps))
            OT = lambda t: (lambda: outproj_t(t, ps))
            # outproj groups run >=1 full block after their normalize so the
            # PE never waits on the recip/broadcast chain; the last block
            # absorbs groups 1 AND 2, leaving only group 3 for the tail.
            sched = [
                dict(b=(0, 1), f=F_DVE_MAIN),
                dict(b=(0, 2), f=F_DVE_MAIN, projs=[K(1, 0), Q(1, 0), K(1, 1)]),
                dict(b=(0, 3), f=F_DVE_MAIN, projs=[Q(1, 1), K(1, 2), Q(1, 2)]),
                dict(b=(1, 0), f=F_DVE_MAIN, projs=[K(1, 3), Q(1, 3)]),
                dict(b=(1, 1), f=F_DVE_MAIN, projs=[K(2, 0), Q(2, 0)]),
                dict(b=(1, 2), f=F_DVE_MAIN, projs=[K(2, 1), Q(2, 1)]),
                dict(b=(1, 3), f=F_DVE_MAIN, projs=[K(2, 2), Q(2, 2)]),
                dict(b=(2, 0), f=F_DVE_OUT, projs=[K(2, 3), Q(2, 3)]),
                dict(b=(2, 1), f=F_DVE_OUT),
                dict(b=(2, 2), f=F_DVE_OUT,
                     outs=[(4, OT(0)), (8, OT(1)), (12, OT(2)), (14, OT(3))]),
                dict(b=(2, 3), f=F_DVE_OUT, fast_norm=True,
                     outs=[(4, OT(4)), (6, OT(5)), (8, OT(6)), (10, OT(7)),
                           (12, OT(8)), (12, OT(9)), (14, OT(10)), (14, OT(11))]),
            ]
            pre = None
            for i, s in enumerate(sched):
                nxt = sched[i + 1] if i + 1 < len(sched) else None
                pre = attn_block(s["b"][0], s["b"][1], ps, cpsp, s["f"],
                                 projs=s.get("projs", ()), outs=s.get("outs", ()),
                                 pre=pre,
                                 nxt=nxt["b"] if nxt else None,
                                 f_nxt=nxt["f"] if nxt else None,
                                 fast_norm=s.get("fast_norm", False))
            for t_i in range(12, 16):
                outproj_t(t_i, ps, ob_act=(t_i % 2 == 0))

    with tile.TileContext(nc) as tc, ExitStack() as ctx:
        consts = ctx.enter_context(tc.tile_pool(name="consts", bufs=1))
        ident_f32 = consts.tile([P, P], mybir.dt.float32)
        make_identity(nc, ident_f32)
        ident_bf = consts.tile([P, P], BF16)
        nc.vector.tensor_copy(ident_bf[:], ident_f32[:])

        big = ctx.enter_context(tc.tile_pool(name="big", bufs=12))
        wrp = ctx.enter_context(tc.tile_pool(name="wrp", bufs=1))
        vap = ctx.enter_context(tc.tile_pool(name="vap", bufs=1))
        work = ctx.enter_context(tc.tile_pool(name="work", bufs=20))
        outp = ctx.enter_context(tc.tile_pool(name="outp", bufs=2))
        norm = ctx.enter_context(tc.tile_pool(name="norm", bufs=2))
        pools = (ident_bf, big, wrp, vap, work, outp, norm)
        for rep in range(repeat):
            emit(f"r{rep}_", tc, pools)

    nc.compile()
    return nc


def kernel(X, Wq, Wk, Wv, Wo, bo):
    from concourse import bass_utils

    if "nc" not in _cache:
        _cache["nc"] = _build(int(os.environ.get("KERNEL_REPEAT", "1")))
    nc = _cache["nc"]

    X = np.asarray(X, dtype=np.float32)
    in_maps = []
    for c in range(8):
        b, g = divmod(c, 2)
        sl = slice(HD * g, HD * (g + 1))
        in_maps.append({
            "x": np.ascontiguousarray(X[b]),
            "wq": np.ascontiguousarray(np.asarray(Wq, np.float32)[:, sl]),
            "wk": np.ascontiguousarray(np.asarray(Wk, np.float32)[:, sl]),
            "wv": np.ascontiguousarray(np.asarray(Wv, np.float32)[:, sl]),
            "wo": np.ascontiguousarray(np.asarray(Wo, np.float32)[sl, :]),
        })
    res = bass_utils.run_bass_kernel_spmd(nc, in_maps, core_ids=list(range(8)))
    _cache["last_result"] = res
    outf = np.empty((4, T, C), np.float32)
    bo = np.asarray(bo, np.float32)
    for b in range(4):
        outf[b] = res.results[2 * b]["out"] + res.results[2 * b + 1]["out"] + bo
    return outf
